# revision 20
# baseline (speedup 1.0000x reference)
"""Trainium2 Bass kernel for a causal self-attention transformer block.

Reference computation (per batch b):
    qkv = x @ w_qkv.T ; split into q, k, v heads (16 heads, dim 64)
    s   = (q @ k.T) * dh**-0.5, causal + padding mask
    a   = softmax(s, axis=j)
    o   = (a @ v) @ w_out.T + b_out ; out = o * m[:, None]

Sharding: pure data parallel — batch (8) across the 8 NeuronCores, weights
replicated. No collectives.

Per-core device program (v2 — see kernel_baseline.py for the v1 notes):
  - inputs host-pre-transposed so every matmul contraction dim (the
    partition dim) needs no on-chip transpose; operands fp16, fp32 PSUM.
  - scores computed transposed S_T[j, i] per head, the two heads' K=64
    matmuls dual-issued in array row-groups 0-1 / 2-3 at full aggregate
    rate (measured 216 ns per 2x512-col pair).
  - v2 changes, driven by the v1 trace (221991 ns):
    * startup: the first matmul sat at t=12.3us because all input DMAs
      were issued from only 2 engine queues (~0.65us sequencer issue
      each) behind a 6.8us framework preamble.  The critical first tiles
      (pair-0 q/k weights + xT d0/d1 halves) are now the FIRST issue on
      4 different engine queues (sync/scalar/vector/gpsimd), with the
      rest of xT/wv/wo/bias spread behind them in consumption order.
    * exp fusion: score PSUM tiles are [128, 2, 512] (2 banks, one per
      head) and pt tiles [128, 2, T], so ONE ACT instruction exps both
      heads' chunk ([128, 2, w]) — halves ACT per-instruction overhead
      (~293 ns each) and halves the PSUM-free events the next score
      burst waits on (v1 lost ~0.3-0.6us/pair to h64 matmuls stalling
      on exp drain).
    * projections emit half-by-half (8 matmuls -> evac) instead of
      interleaving the two PSUM halves, so a bank frees 1.7us earlier
      and the next dest never waits on an evac.
    * bias matmuls removed from the out projection (8192 PE columns):
      bias is added by DVE during the evac against a host-uploaded
      [128, 1024] broadcast of b_out.
    * output stored fp16 (upcast on host) — halves the tail store.
  - the PE stream is organized to minimize array tiling-mode switches
    (each 64-row <-> 128-row mode change drains the array, ~94 ns):
    bursts of 2 score chunks alternate with ~6 matmuls of 128-row filler
    (previous pair's A@V, next pair's q/k projection, normalize of pair
    g-3), sized so ACT exp can drain the 2 score-PSUM tiles in time.
  - normalization per head-pair: denominator row (from the A@V mask
    column) -> DMA-reshaped [128, 8] halves -> DVE reciprocal -> fp16 ->
    rows 0-1 of a zero-padded [128, T] operand; a K=64 matmul against a
    0/1 selector broadcasts it into PSUM, then one in-place multiply on
    the o^T tile.
"""

import os
import numpy as np
from contextlib import ExitStack

import ml_dtypes
from concourse import bacc
import concourse.mybir as mybir
import concourse.tile as tile
from concourse.bass_utils import run_bass_kernel_spmd

D = 1024          # model dim
T = 1024          # sequence length
H = 16            # heads
DH = 64           # head dim
P = 128           # partitions
ND = D // P       # d-tiles
NT = T // P       # t-tiles
NPAIR = H // 2    # head pairs
SCALE = DH ** -0.5
F32 = mybir.dt.float32
F16 = mybir.dt.float16
MULT = mybir.AluOpType.mult
ADD = mybir.AluOpType.add
EXP = mybir.ActivationFunctionType.Exp

MM_DT = mybir.dt.float16
NP_MM = np.float16

_CACHE = {}
LAST_RESULTS = None


def _qk_chunks(J):
    """i-column chunks (lo, width) of computed scores for j-tile J."""
    out = []
    for lo in (J * P, J * P + 512):
        w = min(512, T - lo)
        if w > 0:
            out.append((lo, w))
    return out


def _emit(nc, tc, xT_d, wqk_d, wv_d, wo_d, bbc_d, mcol_d, tri_d,
          sel2_d, out_d):
    ctx = ExitStack()
    with ctx:
        const = ctx.enter_context(tc.tile_pool(name="const", bufs=1))
        xt_p = ctx.enter_context(tc.tile_pool(name="xt", bufs=1))
        vaug_p = ctx.enter_context(tc.tile_pool(name="vaug", bufs=1))
        qkT_p = ctx.enter_context(tc.tile_pool(name="qkT", bufs=2))
        wqk_p = ctx.enter_context(tc.tile_pool(name="wqk", bufs=4))
        pt_p = ctx.enter_context(tc.tile_pool(name="pt", bufs=16))
        oT_p = ctx.enter_context(tc.tile_pool(name="oT", bufs=1))
        wv_p = ctx.enter_context(tc.tile_pool(name="wv", bufs=1))
        wo_p = ctx.enter_context(tc.tile_pool(name="wo", bufs=1))
        osb_p = ctx.enter_context(tc.tile_pool(name="osb", bufs=4))
        osb16_p = ctx.enter_context(tc.tile_pool(name="osb16", bufs=4))
        den_p = ctx.enter_context(tc.tile_pool(name="den", bufs=2))
        psA = ctx.enter_context(tc.tile_pool(name="psA", bufs=2, space="PSUM"))
        psS = ctx.enter_context(tc.tile_pool(name="psS", bufs=2, space="PSUM"))
        psV = ctx.enter_context(tc.tile_pool(name="psV", bufs=2, space="PSUM"))

        # ---- startup load. v1 lesson: the pacers are the per-engine
        # sequencer DMA-issue rate (~0.65us per dma_start) and the ~6.8us
        # framework preamble; transfer bandwidth is plentiful (~270 B/ns
        # aggregate, each dma is sharded over 16 HW DMA engines).  So the
        # first matmul's inputs must be the FIRST issue on their queues,
        # spread over 4 engines (tensor stays DMA-free so nothing delays
        # its first LDWEIGHTS).
        xt_all = xt_p.tile([P, ND, T], MM_DT, tag="xt", name="xt")
        xT_r = xT_d.ap().rearrange("(n p) t -> p n t", p=P)
        wv_all = wv_p.tile([P, ND, T], MM_DT, tag="wv", name="wvt")
        wv_r = wv_d.ap().rearrange("(n p) t -> p n t", p=P)
        wo_all = wo_p.tile([P, NPAIR, T], MM_DT, tag="wo", name="wot")
        wo_r = wo_d.ap().rearrange("(n p) t -> p n t", p=P)
        tri = const.tile([P, P], MM_DT, tag="tri", name="tri")
        mcol = const.tile([P, NT], F32, tag="mcol", name="mcol")
        sel2 = const.tile([P, P], MM_DT, tag="sel2", name="sel2")
        bbc = const.tile([P, D], F32, tag="bbc", name="bbc")

        wts0 = {
            0: wqk_p.tile([P, ND, P], MM_DT, tag="wqk", name="wqt0"),
            NPAIR: wqk_p.tile([P, ND, P], MM_DT, tag="wqk", name="wqtk"),
        }

        def wt0_dma(eng, et, c):
            eng.dma_start(
                out=wts0[et][:, 4 * c:4 * c + 4, :],
                in_=wqk_d.ap()[et][:, 4 * c:4 * c + 4, :],
            )

        def xt_dma(eng, d, h):
            eng.dma_start(
                out=xt_all[:, d:d + 1, h * 512:(h + 1) * 512],
                in_=xT_r[:, d:d + 1, h * 512:(h + 1) * 512],
            )

        # Startup pacing model (v2/v3 traces): sync/scalar queues start
        # issuing at ~6.8us, gpsimd at ~7.6us; ~0.7us per issue; each
        # queue's transfers complete roughly in order at the shared
        # ~250 B/ns aggregate.  So spread the tiles round-robin across
        # the three queues in proj0 CONSUMPTION order (q-half0 d0..d7
        # with its weights, then k-half0, then the h1 halves), with the
        # late bulk (wv/wo/bias) strictly behind.  scalar's (= ACT's)
        # queue stays short so pair-0 exps aren't stuck behind DMA
        # issues.
        xt_dma(nc.sync, 0, 0)
        wt0_dma(nc.scalar, 0, 0)
        xt_dma(nc.gpsimd, 1, 0)
        wt0_dma(nc.sync, 0, 1)
        wt0_dma(nc.scalar, NPAIR, 0)
        wt0_dma(nc.gpsimd, NPAIR, 1)
        xt_dma(nc.sync, 2, 0)
        xt_dma(nc.scalar, 3, 0)
        xt_dma(nc.gpsimd, 0, 1)
        xt_dma(nc.sync, 4, 0)
        xt_dma(nc.scalar, 5, 0)
        xt_dma(nc.gpsimd, 1, 1)
        xt_dma(nc.sync, 6, 0)
        xt_dma(nc.scalar, 7, 0)
        xt_dma(nc.gpsimd, 2, 1)
        nc.scalar.dma_start(out=tri[:], in_=tri_d.ap())
        xt_dma(nc.gpsimd, 3, 1)
        xt_dma(nc.sync, 4, 1)
        xt_dma(nc.sync, 5, 1)
        xt_dma(nc.gpsimd, 6, 1)
        xt_dma(nc.gpsimd, 7, 1)
        for q in range(ND):
            nc.sync.dma_start(
                out=wv_all[:, q:q + 1, :], in_=wv_r[:, q:q + 1, :]
            )
        nc.gpsimd.dma_start(out=mcol[:], in_=mcol_d.ap())
        nc.gpsimd.dma_start(out=sel2[:], in_=sel2_d.ap())
        for q in range(4):
            nc.gpsimd.dma_start(
                out=wo_all[:, 2 * q:2 * q + 2, :],
                in_=wo_r[:, 2 * q:2 * q + 2, :],
            )
        nc.gpsimd.dma_start(out=bbc[:], in_=bbc_d.ap())

        xts = [xt_all[:, d, :] for d in range(ND)]
        wvts = [wv_all[:, d, :] for d in range(ND)]
        wots = [wo_all[:, g, :] for g in range(NPAIR)]

        # v_aug tiles [128 t, 16 h, 65]: per-head v columns * mask + mask col
        vaug = [
            vaug_p.tile([P, H, DH + 1], MM_DT, tag=f"va{t}", name=f"va{t}")
            for t in range(NT)
        ]

        # ---- V projection, as a generator of ~2-MM units woven into
        # pair 0's attention stream.
        def vproj_steps():
            for g2 in range(0, NT, 2):
                accs = {}
                for i in range(2):
                    for c in range(2):
                        pool = psA if i == 0 else psV
                        accs[i, c] = pool.tile(
                            [P, 512], F32, tag=("ps" if i == 0 else "av"),
                            name=f"vps{i}{c}",
                        )
                for d in range(ND):
                    for i in range(2):
                        tt = g2 + i
                        for c in range(2):
                            nc.tensor.matmul(
                                accs[i, c][:],
                                xts[d][:, tt * P:(tt + 1) * P],
                                wvts[d][:, c * 512:(c + 1) * 512],
                                start=(d == 0),
                                stop=(d == ND - 1),
                            )
                        yield
                for i in range(2):
                    tt = g2 + i
                    for c in range(2):
                        ps3 = accs[i, c][:].rearrange("p (h e) -> p h e", e=DH)
                        nc.vector.tensor_scalar(
                            vaug[tt][:, c * 8:(c + 1) * 8, 0:DH],
                            ps3,
                            mcol[:, tt:tt + 1],
                            None,
                            MULT,
                        )
                    nc.vector.tensor_copy(
                        out=vaug[tt][:, :, DH],
                        in_=mcol[:, tt:tt + 1].to_broadcast([P, H]),
                    )
                    yield

        # ---- per-pair building blocks (generators yielding ~1-MM units)
        def _normalize(oT, rcpg):
            # K=64 matmul (sel2 zero-padded to 64 rows) keeps the PE in a
            # full-rate mode without a K=2 32-row switch.
            for c in range(2):
                bc = psV.tile([P, 512], F32, tag="av", name="bc")
                nc.tensor.matmul(
                    bc[:],
                    sel2[:],
                    rcpg[:, c * 512:(c + 1) * 512],
                    start=True, stop=True,
                )
                nc.vector.tensor_tensor(
                    oT[:, c * 512:(c + 1) * 512],
                    oT[:, c * 512:(c + 1) * 512],
                    bc[:],
                    MULT,
                )
                yield

        def proj_dma(g):
            wts = {}
            for et in (g, NPAIR + g):
                wt = wqk_p.tile([P, ND, P], MM_DT, tag="wqk", name="wqkt")
                nc.sync.dma_start(out=wt[:], in_=wqk_d.ap()[et])
                wts[et] = wt
            return wts

        def _proj(g, qT, kT, wts):
            """Pair g's q/k projection, half-by-half: 8 matmuls into one
            PSUM bank then its evac, so the bank frees 1.7us earlier and
            the next half/dest never stalls on the copy."""
            for dest, et in ((qT, g), (kT, NPAIR + g)):
                wt = wts[et]
                for h in range(2):
                    psh = psA.tile([P, 512], F32, tag="ps", name=f"qk{h}")
                    for d0 in range(0, ND, 2):
                        for d in (d0, d0 + 1):
                            nc.tensor.matmul(
                                psh[:], wt[:, d, :],
                                xts[d][:, h * 512:(h + 1) * 512],
                                start=(d == 0), stop=(d == ND - 1),
                            )
                        yield
                    nc.vector.tensor_copy(
                        out=dest[:, h * 512:(h + 1) * 512], in_=psh[:]
                    )
                    yield

        def _pull(it, n):
            for _ in range(n):
                try:
                    next(it)
                except StopIteration:
                    return

        def _pull_n(it, n):
            k = 0
            for _ in range(n):
                try:
                    next(it)
                    k += 1
                except StopIteration:
                    break
            return k

        def _chain(*gens):
            for gg in gens:
                yield from gg

        # two persistent ping-pong buffers for the K-padded reciprocal
        # operand: rows 2..127 zeroed once; each pair's DMA rewrites
        # rows 0-1 of its g%2 buffer.
        rcp_bufs = [
            den_p.tile([P, T], MM_DT, tag=f"rcp{i}", bufs=1, name=f"rcpb{i}")
            for i in range(2)
        ]
        for rb in rcp_bufs:
            nc.gpsimd.memset(rb[:, :], 0.0)

        def _proj0(qT, kT, wts):
            # pair 0 only: half-by-half in xT DMA-arrival order.
            for h in range(2):
                for dest, et in ((qT, 0), (kT, NPAIR)):
                    psh = psA.tile([P, 512], F32, tag="ps", name=f"qk0_{h}")
                    for d in range(ND):
                        nc.tensor.matmul(
                            psh[:], wts[et][:, d, :],
                            xts[d][:, h * 512:(h + 1) * 512],
                            start=(d == 0), stop=(d == ND - 1),
                        )
                    nc.vector.tensor_copy(
                        out=dest[:, h * 512:(h + 1) * 512], in_=psh[:]
                    )

        oTs = []
        qkTs = {0: (
            qkT_p.tile([P, T], MM_DT, tag="qT", name="qT0"),
            qkT_p.tile([P, T], MM_DT, tag="kT", name="kT0"),
        )}
        _proj0(*qkTs[0], wts0)

        op_accs = None

        def _op_steps():
            # first out-proj t-tile, pairs 0..5 (already normalized):
            # weave source for pair 7's attention
            for gg in range(6):
                for c in range(2):
                    nc.tensor.matmul(
                        op_accs[c][:],
                        oTs[gg][:, 0:P],
                        wots[gg][:, c * 512:(c + 1) * 512],
                        start=(gg == 0), stop=False,
                    )
                yield

        pair_pts = {}
        dengs = {}

        def score_steps(g, qT, kT, pts):
            # One unit per score chunk: the two heads' K=64 matmuls occupy
            # array row-groups 0-1 / 2-3 (partition base 0 / 64) and stream
            # concurrently into the two banks of one [128, 2, 512] PSUM
            # tile; ONE fused exp drains both heads' chunk.
            for J in range(NT):
                ptt = pt_p.tile([P, 2, T], MM_DT, tag="pt", name=f"p{g}_{J}")
                pts.append(ptt)
                first = True
                for (lo, w) in _qk_chunks(J):
                    sps = psS.tile([P, 2, 512], F32, tag="s", name="sps")
                    for hh in (0, 1):
                        hs = slice(hh * DH, (hh + 1) * DH)
                        nc.tensor.matmul(
                            sps[:, hh, 0:w],
                            kT[hs, J * P:(J + 1) * P],
                            qT[hs, lo:lo + w],
                            start=True, stop=True,
                        )
                    nc.scalar.activation(
                        out=ptt[:, :, lo:lo + w], in_=sps[:, :, 0:w],
                        func=EXP, scale=SCALE,
                    )
                    if first:
                        # causal mask on the diagonal block (inside chunk 0)
                        for hh in (0, 1):
                            nc.vector.tensor_tensor(
                                ptt[:, hh, J * P:(J + 1) * P],
                                ptt[:, hh, J * P:(J + 1) * P],
                                tri[:],
                                MULT,
                            )
                        first = False
                    yield

        def av_ci(g, ci):
            # A @ V (+ denominator row via the mask column of v_aug) for
            # one 512-column output half, both heads; one unit per matmul.
            # Pair 7 (the tail, no exps left) evacuates through ACT so
            # the DVE queue is free for the reciprocal chain — otherwise
            # norm(7) lands ~3us late and the PE goes idle + HAM-cold.
            pts = pair_pts[g]
            oT = oTs[g]
            deng = dengs[g]
            tail = (g == NPAIR - 1)
            clo, cw = (0, 512) if ci == 0 else (512, 512)
            jmax = 4 if ci == 0 else 8
            for hh in (0, 1):
                h = 2 * g + hh
                av = psV.tile([P, 512], F32, tag="av", name="avps")
                for J in range(jmax):
                    lo = max(clo, J * P)
                    nc.tensor.matmul(
                        av[0:DH + 1, lo - clo:cw],
                        vaug[J][:, h, :],
                        pts[J][:, hh, lo:clo + cw],
                        start=(J == 0), stop=(J == jmax - 1),
                    )
                    yield
                if tail:
                    nc.scalar.copy(
                        out=deng[0:1, ci, hh, :], in_=av[DH:DH + 1, 0:cw],
                    )
                    nc.scalar.copy(
                        out=oT[hh * DH:(hh + 1) * DH, clo:clo + cw],
                        in_=av[0:DH, 0:cw],
                    )
                else:
                    nc.vector.tensor_copy(
                        out=deng[0:1, ci, hh, :],
                        in_=av[DH:DH + 1, 0:cw],
                    )
                    nc.vector.tensor_copy(
                        out=oT[hh * DH:(hh + 1) * DH, clo:clo + cw],
                        in_=av[0:DH, 0:cw],
                    )
                yield

        def recip_half(g, ci):
            # reciprocal of pair g's denominators for one 512-column half
            # (no PE work, no yields). The [1, 1024] half-row is
            # DMA-reshaped to [128, 8] so the reciprocal uses all DVE
            # lanes; result lands in rows 0-1 of the rcp operand.  The
            # two DMA hops cost ~3us of latency, hidden for pairs 0-6 by
            # the 3-pair normalize slack and for pair 7 by the tt1/tt2
            # out-projection cover blocks.
            deng = dengs[g]
            rcpg = rcp_bufs[g % 2]
            cs = slice(ci * 512, (ci + 1) * 512)
            den128 = den_p.tile([P, 8], F32, tag="den128", bufs=4,
                                name=f"d1_{g}_{ci}")
            rec128 = den_p.tile([P, 8], F32, tag="rec128", bufs=4,
                                name=f"r1_{g}_{ci}")
            rsc = den_p.tile([P, 8], F32, tag="rsc", bufs=4,
                             name=f"rs_{g}_{ci}")
            rech = den_p.tile([P, 8], MM_DT, tag="rech", bufs=4,
                              name=f"rh_{g}_{ci}")
            nc.sync.dma_start(out=den128[:], in_=deng[0:1, ci, :, :])
            nc.vector.reciprocal_approx_accurate(
                out=rec128[:], in_=den128[:], scratch=rsc[:]
            )
            with nc.allow_low_precision(reason="fp16 recip feeds matmul"):
                nc.vector.tensor_copy(out=rech[:], in_=rec128[:])
            nc.sync.dma_start(out=rcpg[0:2, cs], in_=rech[:])
            return
            yield  # pragma: no cover — makes this a generator

        def av_recip(g):
            return _chain(av_ci(g, 0), recip_half(g, 0),
                          av_ci(g, 1), recip_half(g, 1))

        # ---- the pair pipeline. Per pair: bursts of 2 score chunks
        # (64-row array mode) alternate with ~6 units of 128-row filler
        # (previous pair's A@V, next pair's q/k projection, normalize of
        # pair g-3), sized so ACT exp can drain the 2 score-PSUM tiles.
        for g in range(NPAIR):
            qT, kT = qkTs[g]
            oT = oT_p.tile([P, T], MM_DT, tag=f"oT{g}", name=f"oT{g}")
            oTs.append(oT)
            dengs[g] = den_p.tile([1, 2, 2, 512], F32, tag="den", name=f"den{g}")
            pts = []
            pair_pts[g] = pts

            fillers = []
            if g >= 3:
                fillers.append(_normalize(oTs[g - 3], rcp_bufs[(g - 3) % 2]))
            if g == NPAIR - 1:
                fillers.append(_normalize(oTs[5], rcp_bufs[5 % 2]))
                fillers.append(av_recip(6))
                op_accs = {
                    c: psA.tile([P, 512], F32, tag="ps", name=f"ops0_{c}")
                    for c in range(2)
                }
                fillers.append(_op_steps())
                # norm(6) at the tail of pair 7's weave: recip(6) has
                # completed by then, and it takes oTs[6] off the
                # post-pair critical path.
                fillers.append(_normalize(oTs[6], rcp_bufs[0]))
            else:
                if g == 0:
                    fillers.append(vproj_steps())
                qkTs[g + 1] = (
                    qkT_p.tile([P, T], MM_DT, tag="qT", name=f"qT{g + 1}"),
                    qkT_p.tile([P, T], MM_DT, tag="kT", name=f"kT{g + 1}"),
                )
                fillers.append(_proj(g + 1, *qkTs[g + 1], proj_dma(g + 1)))
                if g >= 1:
                    fillers.append(av_recip(g - 1))
            filler = _chain(*fillers)

            sgen = score_steps(g, qT, kT, pts)
            while _pull_n(sgen, 2):
                _pull(filler, 6)
            _pull(filler, 9999)

        # ---- Phase 3: output projection, accumulate over head-pair tiles,
        # bias + mask via DVE during the evac, fp16 store. One [128,2,512]
        # PSUM tile (2 banks) per t-tile, double-buffered from psS.
        def _op_finish(tt, acc_of_c):
            for c in range(2):
                osb = osb_p.tile([P, 512], F32, tag="osb", name="osb")
                nc.vector.tensor_tensor(
                    osb[:], acc_of_c(c), bbc[:, c * 512:(c + 1) * 512], ADD,
                )
                osb16 = osb16_p.tile([P, 512], MM_DT, tag="osb16",
                                     name="osb16")
                # mask multiply on ACT (idle in the op phase; per-
                # partition scale) — halves the tail's DVE chain, which
                # gates psS recycling for the next t-tile's block.
                nc.scalar.mul(osb16[:], osb[:], mcol[:, tt:tt + 1])
                for hh, eng in ((0, nc.sync), (1, nc.scalar)):
                    eng.dma_start(
                        out=out_d.ap()[tt * P:(tt + 1) * P,
                                       c * 512 + hh * 256:
                                       c * 512 + (hh + 1) * 256],
                        in_=osb16[:, hh * 256:(hh + 1) * 256],
                    )

        _pull(av_recip(7), 9999)

        def _op_accs_psS(tt):
            return psS.tile([P, 2, 512], F32, tag="s", name=f"ops{tt}")

        def _op_block(accs, tt, gs, start, stop=False):
            for gg in gs:
                for c in range(2):
                    nc.tensor.matmul(
                        accs[:, c, :],
                        oTs[gg][:, tt * P:(tt + 1) * P],
                        wots[gg][:, c * 512:(c + 1) * 512],
                        start=(start and gg == gs[0]),
                        stop=(stop and gg == gs[-1]),
                    )

        # Tail ordering: av(7)'s denominator -> DMA/DVE reciprocal ->
        # norm(7) is a ~3-4us latency chain; cover it with the tt1/tt2
        # pre-blocks over pairs 0..6 (oTs[6] was normalized inside pair
        # 7's weave) so the PE never idles (an idle >3.4us would also
        # re-throttle HAM to half clock, v2's tail regression).
        for c in range(2):
            nc.tensor.matmul(
                op_accs[c][:], oTs[6][:, 0:P],
                wots[6][:, c * 512:(c + 1) * 512],
                start=False, stop=False,
            )
        accs1 = _op_accs_psS(1)
        _op_block(accs1, 1, list(range(7)), True)
        accs2 = _op_accs_psS(2)
        _op_block(accs2, 2, list(range(7)), True)
        _pull(_normalize(oTs[7], rcp_bufs[1]), 99)
        for c in range(2):
            nc.tensor.matmul(
                op_accs[c][:], oTs[7][:, 0:P],
                wots[7][:, c * 512:(c + 1) * 512],
                start=False, stop=True,
            )
        _op_finish(0, lambda c: op_accs[c][:])
        _op_block(accs1, 1, [7], False, stop=True)
        _op_finish(1, lambda c: accs1[:, c, :])
        _op_block(accs2, 2, [7], False, stop=True)
        _op_finish(2, lambda c: accs2[:, c, :])

        for tt in range(3, NT):
            accs = _op_accs_psS(tt)
            _op_block(accs, tt, list(range(NPAIR)), True, stop=True)
            _op_finish(tt, lambda c, a=accs: a[:, c, :])


def build_nc():
    nc = bacc.Bacc("TRN2", target_bir_lowering=False, debug=False,
                   num_devices=8)
    xT_d = nc.dram_tensor("xT", [D, T], MM_DT, kind="ExternalInput")
    wqk_d = nc.dram_tensor("wqk", [H, P, ND, P], MM_DT, kind="ExternalInput")
    wv_d = nc.dram_tensor("wv", [D, D], MM_DT, kind="ExternalInput")
    wo_d = nc.dram_tensor("wo", [D, D], MM_DT, kind="ExternalInput")
    bbc_d = nc.dram_tensor("bbc", [P, D], F32, kind="ExternalInput")
    mcol_d = nc.dram_tensor("mcol", [P, NT], F32, kind="ExternalInput")
    tri_d = nc.dram_tensor("tri", [P, P], MM_DT, kind="ExternalInput")
    sel2_d = nc.dram_tensor("sel2", [P, P], MM_DT, kind="ExternalInput")
    out_d = nc.dram_tensor("out", [T, D], MM_DT, kind="ExternalOutput")
    with tile.TileContext(nc) as tc:
        _emit(nc, tc, xT_d, wqk_d, wv_d, wo_d, bbc_d, mcol_d, tri_d,
              sel2_d, out_d)
    nc.compile()
    return nc


def _prep_shared(w_qkv, w_out, b_out):
    wqkT = np.ascontiguousarray(w_qkv[:2 * D].T)             # [d, e]
    wqk_tiles = np.ascontiguousarray(
        wqkT.reshape(ND, P, H, P).transpose(2, 1, 0, 3)
    ).astype(NP_MM)                                          # [16, 128, 8, 128]
    wv = np.ascontiguousarray(w_qkv[2 * D:].T).astype(NP_MM)  # [d, ev]
    wo = np.ascontiguousarray(w_out.T).astype(NP_MM)          # [d', e]
    bbc = np.ascontiguousarray(
        np.broadcast_to(b_out[None, :], (P, D))
    ).astype(np.float32)                                      # [128, 1024]
    tri = np.triu(np.ones((P, P), dtype=np.float32)).astype(NP_MM)
    # sel2 zero-padded to K=64 rows so its matmul runs in 64-row mode.
    sel2 = np.zeros((P, P), dtype=np.float32)
    sel2[0, 0:DH] = 1.0
    sel2[1, DH:P] = 1.0
    sel2 = sel2.astype(NP_MM)
    return wqk_tiles, wv, wo, bbc, tri, sel2


def kernel(x, m, w_qkv, w_out, b_out, l=None, **_unused):
    global LAST_RESULTS
    x = np.asarray(x, dtype=np.float32)
    m = np.asarray(m, dtype=np.float32)
    w_qkv = np.asarray(w_qkv, dtype=np.float32)
    w_out = np.asarray(w_out, dtype=np.float32)
    b_out = np.asarray(b_out, dtype=np.float32)

    if "nc" not in _CACHE:
        _CACHE["nc"] = build_nc()
    nc = _CACHE["nc"]

    wqk_tiles, wv, wo, bbc, tri, sel2 = _prep_shared(w_qkv, w_out, b_out)
    in_maps = []
    for b in range(8):
        in_maps.append({
            "xT": np.ascontiguousarray(x[b].T).astype(NP_MM),
            "wqk": wqk_tiles,
            "wv": wv,
            "wo": wo,
            "bbc": bbc,
            "mcol": np.ascontiguousarray(m[b].reshape(NT, P).T),
            "tri": tri,
            "sel2": sel2,
        })

    trace = bool(int(os.environ.get("TRN_TRACE", "0")))
    res = run_bass_kernel_spmd(
        nc, in_maps, core_ids=list(range(8)), trace=trace,
    )
    LAST_RESULTS = res
    out = np.stack([res.results[b]["out"] for b in range(8)], axis=0)
    return out.astype(np.float32)


# revision 25
# speedup vs baseline: 1.0044x; 1.0044x over previous
"""Trainium2 Bass kernel for a causal self-attention transformer block.

Reference computation (per batch b):
    qkv = x @ w_qkv.T ; split into q, k, v heads (16 heads, dim 64)
    s   = (q @ k.T) * dh**-0.5, causal + padding mask
    a   = softmax(s, axis=j)
    o   = (a @ v) @ w_out.T + b_out ; out = o * m[:, None]

Sharding: pure data parallel — batch (8) across the 8 NeuronCores, weights
replicated. No collectives.

Per-core device program (v2 — see kernel_baseline.py for the v1 notes):
  - inputs host-pre-transposed so every matmul contraction dim (the
    partition dim) needs no on-chip transpose; operands fp16, fp32 PSUM.
  - scores computed transposed S_T[j, i] per head, the two heads' K=64
    matmuls dual-issued in array row-groups 0-1 / 2-3 at full aggregate
    rate (measured 216 ns per 2x512-col pair).
  - v2 changes, driven by the v1 trace (221991 ns):
    * startup: the first matmul sat at t=12.3us because all input DMAs
      were issued from only 2 engine queues (~0.65us sequencer issue
      each) behind a 6.8us framework preamble.  The critical first tiles
      (pair-0 q/k weights + xT d0/d1 halves) are now the FIRST issue on
      4 different engine queues (sync/scalar/vector/gpsimd), with the
      rest of xT/wv/wo/bias spread behind them in consumption order.
    * exp fusion: score PSUM tiles are [128, 2, 512] (2 banks, one per
      head) and pt tiles [128, 2, T], so ONE ACT instruction exps both
      heads' chunk ([128, 2, w]) — halves ACT per-instruction overhead
      (~293 ns each) and halves the PSUM-free events the next score
      burst waits on (v1 lost ~0.3-0.6us/pair to h64 matmuls stalling
      on exp drain).
    * projections emit half-by-half (8 matmuls -> evac) instead of
      interleaving the two PSUM halves, so a bank frees 1.7us earlier
      and the next dest never waits on an evac.
    * bias matmuls removed from the out projection (8192 PE columns):
      bias is added by DVE during the evac against a host-uploaded
      [128, 1024] broadcast of b_out.
    * output stored fp16 (upcast on host) — halves the tail store.
  - the PE stream is organized to minimize array tiling-mode switches
    (each 64-row <-> 128-row mode change drains the array, ~94 ns):
    bursts of 2 score chunks alternate with ~6 matmuls of 128-row filler
    (previous pair's A@V, next pair's q/k projection, normalize of pair
    g-3), sized so ACT exp can drain the 2 score-PSUM tiles in time.
  - normalization per head-pair: denominator row (from the A@V mask
    column) -> DMA-reshaped [128, 8] halves -> DVE reciprocal -> fp16 ->
    rows 0-1 of a zero-padded [128, T] operand; a K=64 matmul against a
    0/1 selector broadcasts it into PSUM, then one in-place multiply on
    the o^T tile.
"""

import os
import numpy as np
from contextlib import ExitStack

import ml_dtypes
from concourse import bacc
import concourse.mybir as mybir
import concourse.tile as tile
from concourse.bass_utils import run_bass_kernel_spmd

D = 1024          # model dim
T = 1024          # sequence length
H = 16            # heads
DH = 64           # head dim
P = 128           # partitions
ND = D // P       # d-tiles
NT = T // P       # t-tiles
NPAIR = H // 2    # head pairs
SCALE = DH ** -0.5
F32 = mybir.dt.float32
F16 = mybir.dt.float16
MULT = mybir.AluOpType.mult
ADD = mybir.AluOpType.add
EXP = mybir.ActivationFunctionType.Exp

MM_DT = mybir.dt.float16
NP_MM = np.float16

_CACHE = {}
LAST_RESULTS = None


def _qk_chunks(J):
    """i-column chunks (lo, width) of computed scores for j-tile J."""
    out = []
    for lo in (J * P, J * P + 512):
        w = min(512, T - lo)
        if w > 0:
            out.append((lo, w))
    return out


def _emit(nc, tc, xT_d, wqk_d, wv_d, wo_d, bbc_d, mcol_d, tri_d,
          sel2_d, out_d):
    ctx = ExitStack()
    with ctx:
        const = ctx.enter_context(tc.tile_pool(name="const", bufs=1))
        xt_p = ctx.enter_context(tc.tile_pool(name="xt", bufs=1))
        vaug_p = ctx.enter_context(tc.tile_pool(name="vaug", bufs=1))
        qkT_p = ctx.enter_context(tc.tile_pool(name="qkT", bufs=2))
        wqk_p = ctx.enter_context(tc.tile_pool(name="wqk", bufs=4))
        pt_p = ctx.enter_context(tc.tile_pool(name="pt", bufs=16))
        oT_p = ctx.enter_context(tc.tile_pool(name="oT", bufs=1))
        wv_p = ctx.enter_context(tc.tile_pool(name="wv", bufs=1))
        wo_p = ctx.enter_context(tc.tile_pool(name="wo", bufs=1))
        osb_p = ctx.enter_context(tc.tile_pool(name="osb", bufs=4))
        osb16_p = ctx.enter_context(tc.tile_pool(name="osb16", bufs=4))
        den_p = ctx.enter_context(tc.tile_pool(name="den", bufs=2))
        psA = ctx.enter_context(tc.tile_pool(name="psA", bufs=2, space="PSUM"))
        psS = ctx.enter_context(tc.tile_pool(name="psS", bufs=2, space="PSUM"))
        psV = ctx.enter_context(tc.tile_pool(name="psV", bufs=2, space="PSUM"))

        # ---- startup load. v1 lesson: the pacers are the per-engine
        # sequencer DMA-issue rate (~0.65us per dma_start) and the ~6.8us
        # framework preamble; transfer bandwidth is plentiful (~270 B/ns
        # aggregate, each dma is sharded over 16 HW DMA engines).  So the
        # first matmul's inputs must be the FIRST issue on their queues,
        # spread over 4 engines (tensor stays DMA-free so nothing delays
        # its first LDWEIGHTS).
        xt_all = xt_p.tile([P, ND, T], MM_DT, tag="xt", name="xt")
        xT_r = xT_d.ap().rearrange("(n p) t -> p n t", p=P)
        wv_all = wv_p.tile([P, ND, T], MM_DT, tag="wv", name="wvt")
        wv_r = wv_d.ap().rearrange("(n p) t -> p n t", p=P)
        wo_all = wo_p.tile([P, NPAIR, T], MM_DT, tag="wo", name="wot")
        wo_r = wo_d.ap().rearrange("(n p) t -> p n t", p=P)
        tri = const.tile([P, P], MM_DT, tag="tri", name="tri")
        mcol = const.tile([P, NT], F32, tag="mcol", name="mcol")
        sel2 = const.tile([P, P], MM_DT, tag="sel2", name="sel2")
        bbc = const.tile([P, D], F32, tag="bbc", name="bbc")

        wts0 = {
            0: wqk_p.tile([P, ND, P], MM_DT, tag="wqk", name="wqt0"),
            NPAIR: wqk_p.tile([P, ND, P], MM_DT, tag="wqk", name="wqtk"),
        }

        def wt0_dma(eng, et, c):
            eng.dma_start(
                out=wts0[et][:, 4 * c:4 * c + 4, :],
                in_=wqk_d.ap()[et][:, 4 * c:4 * c + 4, :],
            )

        def xt_dma(eng, d, h):
            eng.dma_start(
                out=xt_all[:, d:d + 1, h * 512:(h + 1) * 512],
                in_=xT_r[:, d:d + 1, h * 512:(h + 1) * 512],
            )

        # Startup pacing model (v2/v3 traces): sync/scalar queues start
        # issuing at ~6.8us, gpsimd at ~7.6us; ~0.7us per issue; each
        # queue's transfers complete roughly in order at the shared
        # ~250 B/ns aggregate.  So spread the tiles round-robin across
        # the three queues in proj0 CONSUMPTION order (q-half0 d0..d7
        # with its weights, then k-half0, then the h1 halves), with the
        # late bulk (wv/wo/bias) strictly behind.  scalar's (= ACT's)
        # queue stays short so pair-0 exps aren't stuck behind DMA
        # issues.
        xt_dma(nc.sync, 0, 0)
        wt0_dma(nc.scalar, 0, 0)
        xt_dma(nc.gpsimd, 1, 0)
        wt0_dma(nc.sync, 0, 1)
        wt0_dma(nc.scalar, NPAIR, 0)
        wt0_dma(nc.gpsimd, NPAIR, 1)
        xt_dma(nc.sync, 2, 0)
        xt_dma(nc.scalar, 3, 0)
        xt_dma(nc.gpsimd, 0, 1)
        xt_dma(nc.sync, 4, 0)
        xt_dma(nc.scalar, 5, 0)
        xt_dma(nc.gpsimd, 1, 1)
        xt_dma(nc.sync, 6, 0)
        xt_dma(nc.scalar, 7, 0)
        xt_dma(nc.gpsimd, 2, 1)
        nc.scalar.dma_start(out=tri[:], in_=tri_d.ap())
        xt_dma(nc.gpsimd, 3, 1)
        xt_dma(nc.sync, 4, 1)
        xt_dma(nc.sync, 5, 1)
        xt_dma(nc.gpsimd, 6, 1)
        xt_dma(nc.gpsimd, 7, 1)
        for q in range(ND):
            nc.sync.dma_start(
                out=wv_all[:, q:q + 1, :], in_=wv_r[:, q:q + 1, :]
            )
        nc.gpsimd.dma_start(out=mcol[:], in_=mcol_d.ap())
        nc.gpsimd.dma_start(out=sel2[:], in_=sel2_d.ap())
        for q in range(4):
            nc.gpsimd.dma_start(
                out=wo_all[:, 2 * q:2 * q + 2, :],
                in_=wo_r[:, 2 * q:2 * q + 2, :],
            )
        nc.gpsimd.dma_start(out=bbc[:], in_=bbc_d.ap())

        xts = [xt_all[:, d, :] for d in range(ND)]
        wvts = [wv_all[:, d, :] for d in range(ND)]
        wots = [wo_all[:, g, :] for g in range(NPAIR)]

        # ---- HAM warmup: the PE clock-gate (K=4/8 -> half clock) only
        # lifts after ~3.4us of sustained PE activity, and the DMA-paced
        # early stream keeps resetting the window (v5 trace: cold until
        # t=20.8us).  8 dummy N=512 matmuls on a memset scratch keep the
        # PE busy from ~6.6us so the real stream runs warm.  They write
        # score-pool PSUM (unused until ~23us) and cost nothing the
        # DMA-starved PE could otherwise do.
        scr = const.tile([P, 512], MM_DT, tag="warm", name="warm")
        nc.vector.memset(scr[:], 0.001)
        for wi in range(4):
            dps = psS.tile([P, 2, 512], F32, tag="s", name=f"warm{wi}")
            for c in range(2):
                nc.tensor.matmul(
                    dps[:, c, :], scr[:, 0:P], scr[:],
                    start=True, stop=True,
                )

        # v_aug tiles [128 t, 16 h, 65]: per-head v columns * mask + mask col
        vaug = [
            vaug_p.tile([P, H, DH + 1], MM_DT, tag=f"va{t}", name=f"va{t}")
            for t in range(NT)
        ]

        # ---- V projection, as a generator of ~2-MM units woven into
        # pair 0's attention stream.
        def vproj_steps():
            for g2 in range(0, NT, 2):
                accs = {}
                for i in range(2):
                    for c in range(2):
                        pool = psA if i == 0 else psV
                        accs[i, c] = pool.tile(
                            [P, 512], F32, tag=("ps" if i == 0 else "av"),
                            name=f"vps{i}{c}",
                        )
                for d in range(ND):
                    for i in range(2):
                        tt = g2 + i
                        for c in range(2):
                            nc.tensor.matmul(
                                accs[i, c][:],
                                xts[d][:, tt * P:(tt + 1) * P],
                                wvts[d][:, c * 512:(c + 1) * 512],
                                start=(d == 0),
                                stop=(d == ND - 1),
                            )
                        yield
                for i in range(2):
                    tt = g2 + i
                    for c in range(2):
                        ps3 = accs[i, c][:].rearrange("p (h e) -> p h e", e=DH)
                        nc.vector.tensor_scalar(
                            vaug[tt][:, c * 8:(c + 1) * 8, 0:DH],
                            ps3,
                            mcol[:, tt:tt + 1],
                            None,
                            MULT,
                        )
                    nc.vector.tensor_copy(
                        out=vaug[tt][:, :, DH],
                        in_=mcol[:, tt:tt + 1].to_broadcast([P, H]),
                    )
                    yield

        # ---- per-pair building blocks (generators yielding ~1-MM units)
        def _normalize(oT, rcpg):
            # K=64 matmul (sel2 zero-padded to 64 rows) keeps the PE in a
            # full-rate mode without a K=2 32-row switch.
            for c in range(2):
                bc = psV.tile([P, 512], F32, tag="av", name="bc")
                nc.tensor.matmul(
                    bc[:],
                    sel2[:],
                    rcpg[:, c * 512:(c + 1) * 512],
                    start=True, stop=True,
                )
                nc.vector.tensor_tensor(
                    oT[:, c * 512:(c + 1) * 512],
                    oT[:, c * 512:(c + 1) * 512],
                    bc[:],
                    MULT,
                )
                yield

        def proj_dma(g):
            wts = {}
            for et in (g, NPAIR + g):
                wt = wqk_p.tile([P, ND, P], MM_DT, tag="wqk", name="wqkt")
                nc.sync.dma_start(out=wt[:], in_=wqk_d.ap()[et])
                wts[et] = wt
            return wts

        def _proj(g, qT, kT, wts):
            """Pair g's q/k projection, half-by-half: 8 matmuls into one
            PSUM bank then its evac, so the bank frees 1.7us earlier and
            the next half/dest never stalls on the copy."""
            for dest, et in ((qT, g), (kT, NPAIR + g)):
                wt = wts[et]
                for h in range(2):
                    psh = psA.tile([P, 512], F32, tag="ps", name=f"qk{h}")
                    for d0 in range(0, ND, 2):
                        for d in (d0, d0 + 1):
                            nc.tensor.matmul(
                                psh[:], wt[:, d, :],
                                xts[d][:, h * 512:(h + 1) * 512],
                                start=(d == 0), stop=(d == ND - 1),
                            )
                        yield
                    nc.vector.tensor_copy(
                        out=dest[:, h * 512:(h + 1) * 512], in_=psh[:]
                    )
                    yield

        def _pull(it, n):
            for _ in range(n):
                try:
                    next(it)
                except StopIteration:
                    return

        def _pull_n(it, n):
            k = 0
            for _ in range(n):
                try:
                    next(it)
                    k += 1
                except StopIteration:
                    break
            return k

        def _chain(*gens):
            for gg in gens:
                yield from gg

        # two persistent ping-pong buffers for the K-padded reciprocal
        # operand: rows 2..127 zeroed once; each pair's DMA rewrites
        # rows 0-1 of its g%2 buffer.
        rcp_bufs = [
            den_p.tile([P, T], MM_DT, tag=f"rcp{i}", bufs=1, name=f"rcpb{i}")
            for i in range(2)
        ]
        for rb in rcp_bufs:
            nc.gpsimd.memset(rb[:, :], 0.0)

        def _proj0(qT, kT, wts):
            # pair 0 only: half-by-half in xT DMA-arrival order.
            for h in range(2):
                for dest, et in ((qT, 0), (kT, NPAIR)):
                    psh = psA.tile([P, 512], F32, tag="ps", name=f"qk0_{h}")
                    for d in range(ND):
                        nc.tensor.matmul(
                            psh[:], wts[et][:, d, :],
                            xts[d][:, h * 512:(h + 1) * 512],
                            start=(d == 0), stop=(d == ND - 1),
                        )
                    nc.vector.tensor_copy(
                        out=dest[:, h * 512:(h + 1) * 512], in_=psh[:]
                    )

        oTs = []
        qkTs = {0: (
            qkT_p.tile([P, T], MM_DT, tag="qT", name="qT0"),
            qkT_p.tile([P, T], MM_DT, tag="kT", name="kT0"),
        )}
        _proj0(*qkTs[0], wts0)

        op_accs = None

        def _op_steps():
            # first out-proj t-tile, pairs 0..5 (already normalized):
            # weave source for pair 7's attention
            for gg in range(6):
                for c in range(2):
                    nc.tensor.matmul(
                        op_accs[c][:],
                        oTs[gg][:, 0:P],
                        wots[gg][:, c * 512:(c + 1) * 512],
                        start=(gg == 0), stop=False,
                    )
                yield

        pair_pts = {}
        dengs = {}

        def score_steps(g, qT, kT, pts):
            # One unit per score chunk: the two heads' K=64 matmuls occupy
            # array row-groups 0-1 / 2-3 (partition base 0 / 64) and stream
            # concurrently into the two banks of one [128, 2, 512] PSUM
            # tile; ONE fused exp drains both heads' chunk.
            for J in range(NT):
                ptt = pt_p.tile([P, 2, T], MM_DT, tag="pt", name=f"p{g}_{J}")
                pts.append(ptt)
                first = True
                for (lo, w) in _qk_chunks(J):
                    sps = psS.tile([P, 2, 512], F32, tag="s", name="sps")
                    for hh in (0, 1):
                        hs = slice(hh * DH, (hh + 1) * DH)
                        nc.tensor.matmul(
                            sps[:, hh, 0:w],
                            kT[hs, J * P:(J + 1) * P],
                            qT[hs, lo:lo + w],
                            start=True, stop=True,
                        )
                    nc.scalar.activation(
                        out=ptt[:, :, lo:lo + w], in_=sps[:, :, 0:w],
                        func=EXP, scale=SCALE,
                    )
                    if first:
                        # causal mask on the diagonal block (inside chunk 0)
                        for hh in (0, 1):
                            nc.vector.tensor_tensor(
                                ptt[:, hh, J * P:(J + 1) * P],
                                ptt[:, hh, J * P:(J + 1) * P],
                                tri[:],
                                MULT,
                            )
                        first = False
                    yield

        def av_ci(g, ci, act_evac=False):
            # A @ V (+ denominator row via the mask column of v_aug) for
            # one 512-column output half, both heads; one unit per matmul.
            # Pair 7's ci1 half (the tail, no exps left) evacuates
            # through ACT so the DVE queue is free for the reciprocal
            # chain — otherwise norm(7) lands ~3us late and the PE goes
            # idle + HAM-cold.
            pts = pair_pts[g]
            oT = oTs[g]
            deng = dengs[g]
            tail = act_evac
            clo, cw = (0, 512) if ci == 0 else (512, 512)
            jmax = 4 if ci == 0 else 8
            for hh in (0, 1):
                h = 2 * g + hh
                av = psV.tile([P, 512], F32, tag="av", name="avps")
                for J in range(jmax):
                    lo = max(clo, J * P)
                    nc.tensor.matmul(
                        av[0:DH + 1, lo - clo:cw],
                        vaug[J][:, h, :],
                        pts[J][:, hh, lo:clo + cw],
                        start=(J == 0), stop=(J == jmax - 1),
                    )
                    yield
                if tail:
                    nc.scalar.copy(
                        out=deng[0:1, ci, hh, :], in_=av[DH:DH + 1, 0:cw],
                    )
                    nc.scalar.copy(
                        out=oT[hh * DH:(hh + 1) * DH, clo:clo + cw],
                        in_=av[0:DH, 0:cw],
                    )
                else:
                    nc.vector.tensor_copy(
                        out=deng[0:1, ci, hh, :],
                        in_=av[DH:DH + 1, 0:cw],
                    )
                    nc.vector.tensor_copy(
                        out=oT[hh * DH:(hh + 1) * DH, clo:clo + cw],
                        in_=av[0:DH, 0:cw],
                    )
                yield

        def recip_half(g, ci):
            # reciprocal of pair g's denominators for one 512-column half
            # (no PE work, no yields). The [1, 1024] half-row is
            # DMA-reshaped to [128, 8] so the reciprocal uses all DVE
            # lanes; result lands in rows 0-1 of the rcp operand.  The
            # two DMA hops cost ~3us of latency, hidden for pairs 0-6 by
            # the 3-pair normalize slack and for pair 7 by the tt1/tt2
            # out-projection cover blocks.
            deng = dengs[g]
            rcpg = rcp_bufs[g % 2]
            cs = slice(ci * 512, (ci + 1) * 512)
            den128 = den_p.tile([P, 8], F32, tag="den128", bufs=4,
                                name=f"d1_{g}_{ci}")
            rec128 = den_p.tile([P, 8], F32, tag="rec128", bufs=4,
                                name=f"r1_{g}_{ci}")
            rsc = den_p.tile([P, 8], F32, tag="rsc", bufs=4,
                             name=f"rs_{g}_{ci}")
            rech = den_p.tile([P, 8], MM_DT, tag="rech", bufs=4,
                              name=f"rh_{g}_{ci}")
            nc.sync.dma_start(out=den128[:], in_=deng[0:1, ci, :, :])
            nc.vector.reciprocal_approx_accurate(
                out=rec128[:], in_=den128[:], scratch=rsc[:]
            )
            with nc.allow_low_precision(reason="fp16 recip feeds matmul"):
                nc.vector.tensor_copy(out=rech[:], in_=rec128[:])
            nc.sync.dma_start(out=rcpg[0:2, cs], in_=rech[:])
            return
            yield  # pragma: no cover — makes this a generator

        def av_recip(g):
            return _chain(av_ci(g, 0), recip_half(g, 0),
                          av_ci(g, 1), recip_half(g, 1))

        # ---- the pair pipeline. Per pair: bursts of 2 score chunks
        # (64-row array mode) alternate with ~6 units of 128-row filler
        # (previous pair's A@V, next pair's q/k projection, normalize of
        # pair g-3), sized so ACT exp can drain the 2 score-PSUM tiles.
        for g in range(NPAIR):
            qT, kT = qkTs[g]
            oT = oT_p.tile([P, T], MM_DT, tag=f"oT{g}", name=f"oT{g}")
            oTs.append(oT)
            dengs[g] = den_p.tile([1, 2, 2, 512], F32, tag="den", name=f"den{g}")
            pts = []
            pair_pts[g] = pts

            fillers = []
            if g >= 3:
                fillers.append(_normalize(oTs[g - 3], rcp_bufs[(g - 3) % 2]))
            if g == NPAIR - 1:
                fillers.append(_normalize(oTs[5], rcp_bufs[5 % 2]))
                fillers.append(av_recip(6))
                op_accs = {
                    c: psA.tile([P, 512], F32, tag="ps", name=f"ops0_{c}")
                    for c in range(2)
                }
                fillers.append(_op_steps())
                # norm(6) at the tail of pair 7's weave: recip(6) has
                # completed by then, and it takes oTs[6] off the
                # post-pair critical path.
                fillers.append(_normalize(oTs[6], rcp_bufs[0]))
            else:
                if g == 0:
                    fillers.append(vproj_steps())
                qkTs[g + 1] = (
                    qkT_p.tile([P, T], MM_DT, tag="qT", name=f"qT{g + 1}"),
                    qkT_p.tile([P, T], MM_DT, tag="kT", name=f"kT{g + 1}"),
                )
                fillers.append(_proj(g + 1, *qkTs[g + 1], proj_dma(g + 1)))
                if g >= 1:
                    fillers.append(av_recip(g - 1))
            filler = _chain(*fillers)

            sgen = score_steps(g, qT, kT, pts)
            if g == NPAIR - 1:
                # Pair 7: after the J<=3 chunks (8 score units), pt(7)'s
                # first four j-tiles are exp'd, so av(7, ci0) + its
                # reciprocal weave INTO the pair — only the ci1 half
                # stays on the tail's critical path.
                for _ in range(4):
                    if not _pull_n(sgen, 2):
                        break
                    _pull(filler, 6)
                filler = _chain(av_ci(7, 0), recip_half(7, 0), filler)
            while _pull_n(sgen, 2):
                _pull(filler, 6)
            _pull(filler, 9999)

        # ---- Phase 3: output projection, accumulate over head-pair tiles,
        # bias + mask via DVE during the evac, fp16 store. One [128,2,512]
        # PSUM tile (2 banks) per t-tile, double-buffered from psS.
        def _op_finish(tt, acc_of_c):
            for c in range(2):
                osb = osb_p.tile([P, 512], F32, tag="osb", name="osb")
                nc.vector.tensor_tensor(
                    osb[:], acc_of_c(c), bbc[:, c * 512:(c + 1) * 512], ADD,
                )
                osb16 = osb16_p.tile([P, 512], MM_DT, tag="osb16",
                                     name="osb16")
                # mask multiply on ACT (idle in the op phase; per-
                # partition scale) — halves the tail's DVE chain, which
                # gates psS recycling for the next t-tile's block.
                nc.scalar.mul(osb16[:], osb[:], mcol[:, tt:tt + 1])
                for hh, eng in ((0, nc.sync), (1, nc.scalar)):
                    eng.dma_start(
                        out=out_d.ap()[tt * P:(tt + 1) * P,
                                       c * 512 + hh * 256:
                                       c * 512 + (hh + 1) * 256],
                        in_=osb16[:, hh * 256:(hh + 1) * 256],
                    )

        def _op_accs_psS(tt):
            return psS.tile([P, 2, 512], F32, tag="s", name=f"ops{tt}")

        def _op_block(accs, tt, gs, start, stop=False):
            for gg in gs:
                for c in range(2):
                    nc.tensor.matmul(
                        accs[:, c, :],
                        oTs[gg][:, tt * P:(tt + 1) * P],
                        wots[gg][:, c * 512:(c + 1) * 512],
                        start=(start and gg == gs[0]),
                        stop=(stop and gg == gs[-1]),
                    )

        # Tail ordering: only av(7, ci1) -> reciprocal -> norm(7) c1
        # remains latency-critical (ci0 was woven into pair 7); cover it
        # with the tt1/tt2 pre-blocks over pairs 0..6 (oTs[6] was
        # normalized inside pair 7's weave) so the PE never idles (an
        # idle >3.4us would also re-throttle HAM to half clock).
        _pull(_chain(av_ci(7, 1, act_evac=True), recip_half(7, 1)), 9999)
        for c in range(2):
            nc.tensor.matmul(
                op_accs[c][:], oTs[6][:, 0:P],
                wots[6][:, c * 512:(c + 1) * 512],
                start=False, stop=False,
            )
        accs1 = _op_accs_psS(1)
        _op_block(accs1, 1, list(range(7)), True)
        accs2 = _op_accs_psS(2)
        _op_block(accs2, 2, list(range(7)), True)
        _pull(_normalize(oTs[7], rcp_bufs[1]), 99)
        for c in range(2):
            nc.tensor.matmul(
                op_accs[c][:], oTs[7][:, 0:P],
                wots[7][:, c * 512:(c + 1) * 512],
                start=False, stop=True,
            )
        _op_finish(0, lambda c: op_accs[c][:])
        _op_block(accs1, 1, [7], False, stop=True)
        _op_finish(1, lambda c: accs1[:, c, :])
        _op_block(accs2, 2, [7], False, stop=True)
        _op_finish(2, lambda c: accs2[:, c, :])

        for tt in range(3, NT):
            accs = _op_accs_psS(tt)
            _op_block(accs, tt, list(range(NPAIR)), True, stop=True)
            _op_finish(tt, lambda c, a=accs: a[:, c, :])


def build_nc():
    nc = bacc.Bacc("TRN2", target_bir_lowering=False, debug=False,
                   num_devices=8)
    xT_d = nc.dram_tensor("xT", [D, T], MM_DT, kind="ExternalInput")
    wqk_d = nc.dram_tensor("wqk", [H, P, ND, P], MM_DT, kind="ExternalInput")
    wv_d = nc.dram_tensor("wv", [D, D], MM_DT, kind="ExternalInput")
    wo_d = nc.dram_tensor("wo", [D, D], MM_DT, kind="ExternalInput")
    bbc_d = nc.dram_tensor("bbc", [P, D], F32, kind="ExternalInput")
    mcol_d = nc.dram_tensor("mcol", [P, NT], F32, kind="ExternalInput")
    tri_d = nc.dram_tensor("tri", [P, P], MM_DT, kind="ExternalInput")
    sel2_d = nc.dram_tensor("sel2", [P, P], MM_DT, kind="ExternalInput")
    out_d = nc.dram_tensor("out", [T, D], MM_DT, kind="ExternalOutput")
    with tile.TileContext(nc) as tc:
        _emit(nc, tc, xT_d, wqk_d, wv_d, wo_d, bbc_d, mcol_d, tri_d,
              sel2_d, out_d)
    nc.compile()
    return nc


def _prep_shared(w_qkv, w_out, b_out):
    wqkT = np.ascontiguousarray(w_qkv[:2 * D].T)             # [d, e]
    wqk_tiles = np.ascontiguousarray(
        wqkT.reshape(ND, P, H, P).transpose(2, 1, 0, 3)
    ).astype(NP_MM)                                          # [16, 128, 8, 128]
    wv = np.ascontiguousarray(w_qkv[2 * D:].T).astype(NP_MM)  # [d, ev]
    wo = np.ascontiguousarray(w_out.T).astype(NP_MM)          # [d', e]
    bbc = np.ascontiguousarray(
        np.broadcast_to(b_out[None, :], (P, D))
    ).astype(np.float32)                                      # [128, 1024]
    tri = np.triu(np.ones((P, P), dtype=np.float32)).astype(NP_MM)
    # sel2 zero-padded to K=64 rows so its matmul runs in 64-row mode.
    sel2 = np.zeros((P, P), dtype=np.float32)
    sel2[0, 0:DH] = 1.0
    sel2[1, DH:P] = 1.0
    sel2 = sel2.astype(NP_MM)
    return wqk_tiles, wv, wo, bbc, tri, sel2


def kernel(x, m, w_qkv, w_out, b_out, l=None, **_unused):
    global LAST_RESULTS
    x = np.asarray(x, dtype=np.float32)
    m = np.asarray(m, dtype=np.float32)
    w_qkv = np.asarray(w_qkv, dtype=np.float32)
    w_out = np.asarray(w_out, dtype=np.float32)
    b_out = np.asarray(b_out, dtype=np.float32)

    if "nc" not in _CACHE:
        _CACHE["nc"] = build_nc()
    nc = _CACHE["nc"]

    wqk_tiles, wv, wo, bbc, tri, sel2 = _prep_shared(w_qkv, w_out, b_out)
    in_maps = []
    for b in range(8):
        in_maps.append({
            "xT": np.ascontiguousarray(x[b].T).astype(NP_MM),
            "wqk": wqk_tiles,
            "wv": wv,
            "wo": wo,
            "bbc": bbc,
            "mcol": np.ascontiguousarray(m[b].reshape(NT, P).T),
            "tri": tri,
            "sel2": sel2,
        })

    trace = bool(int(os.environ.get("TRN_TRACE", "0")))
    res = run_bass_kernel_spmd(
        nc, in_maps, core_ids=list(range(8)), trace=trace,
    )
    LAST_RESULTS = res
    out = np.stack([res.results[b]["out"] for b in range(8)], axis=0)
    return out.astype(np.float32)
